# revision 1
# baseline (speedup 1.0000x reference)
# Trainium2 Bass kernel for AoE-style MoE (dense formulation).
#
# Problem: E=8 experts, top-K=2, H=1024, F=2048, low-rank gate R=64,
# tokens N = 2*2048 = 4096.  Reference computes every expert densely with
# zero combine-weight for unselected experts; we do the same, sharding the
# token axis across 8 NeuronCores (data parallel, no collectives).
#
# Per core (T=512 tokens):
#   gate:  gh_e = W_A[e] @ x  (fp32 matmuls, exact top-2 selection)
#          ssq  = sum_r gh^2  -> token-major scores via selector matmul
#          top-2 + softmax on DVE/ACT, weights w[t,e]
#          w broadcast across partitions via selector matmul, folded into x
#   main:  up_e = W_up[e] @ (x*w_e)   (bf16)
#          g_e  = W_B[e] @ gh_e       (bf16)
#          h_e  = silu(g_e) * up_e    (bf16)
#          y   += W_down[e].T-contract h_e   (token-major PSUM, fp32 accum)
#
# kernel(**inputs) takes full unsharded inputs, returns full output.

import os
import sys
import types
import numpy as np
import ml_dtypes

E, TOPK, H, F, R = 8, 2, 1024, 2048, 64
B, S = 2, 2048
N = B * S            # 4096 tokens
NCORES = 8
T = N // NCORES      # 512 tokens per core
TG = 256             # token group for up/g matmuls
FCH = 1024           # F chunk (half of F) streamed per weight DMA

BF16 = ml_dtypes.bfloat16


def _maybe_install_trace_hook():
    """Install the axon NTFF profiling hook if requested and available."""
    if os.environ.get("MOE_TRACE") != "1":
        return False
    try:
        import antenv.axon_hooks  # noqa: F401
        return True
    except ImportError:
        pass
    try:
        if "/root/.axon_site" not in sys.path:
            sys.path.insert(0, "/root/.axon_site")
        from trn_agent_boot.trn_boot import _ntff_profile_via_ctypes
        hook = _ntff_profile_via_ctypes("/opt/axon/libaxon_pjrt.so")
        mod = types.ModuleType("antenv.axon_hooks")
        mod.get_axon_ntff_profile_hook = lambda: hook
        mod.set_axon_ntff_profile_hook = lambda h: None
        sys.modules["antenv.axon_hooks"] = mod
        return True
    except Exception:
        return False


_NC_CACHE = {}
LAST_RESULT = None  # BassKernelResults of the most recent run (for profiling)


def _build_nc():
    import concourse.mybir as mybir
    import concourse.tile as tile
    from concourse import bacc

    f32 = mybir.dt.float32
    bf16 = mybir.dt.bfloat16
    AF = mybir.ActivationFunctionType
    OP = mybir.AluOpType
    AX = mybir.AxisListType

    nc = bacc.Bacc("TRN2", target_bir_lowering=False, debug=False,
                   num_devices=NCORES)

    xT_d = nc.dram_tensor("xT", [128, 8, T], f32, kind="ExternalInput")
    xTbf_d = nc.dram_tensor("xTbf", [128, 8, T], bf16, kind="ExternalInput")
    WA_d = nc.dram_tensor("WA", [128, E, 8, R], f32, kind="ExternalInput")
    WB_d = nc.dram_tensor("WB", [E, 128, F], bf16, kind="ExternalInput")
    WUP_d = nc.dram_tensor("WUP", [E, 128, 8, F], bf16, kind="ExternalInput")
    WDN_d = nc.dram_tensor("WDN", [E, 128, 16, H], bf16, kind="ExternalInput")
    esel_d = nc.dram_tensor("esel", [128, E, E], f32, kind="ExternalInput")
    bsel_d = nc.dram_tensor("bsel", [8, E, 128], f32, kind="ExternalInput")
    ident_d = nc.dram_tensor("ident", [128, 128], f32, kind="ExternalInput")
    out_d = nc.dram_tensor("out", [128, 4, H], f32, kind="ExternalOutput")

    with tile.TileContext(nc) as tc:
        from contextlib import ExitStack
        with ExitStack() as ctx:
            pp = ctx.enter_context(tc.tile_pool(name="persist", bufs=1))

            # persistent SBUF tensors
            xTbf = pp.tile([128, 8, T], bf16, tag="xTbf")
            nc.sync.dma_start(xTbf[:], xTbf_d[:])
            gh_bf = pp.tile([128, E, T], bf16, tag="gh_bf")
            nc.vector.memset(gh_bf[:], 0.0)
            w_bc = pp.tile([128, E, T], bf16, tag="w_bc")
            yT = pp.tile([128, 4, H], f32, tag="yT")
            nc.vector.memset(yT[:], 0.0)

            # ---------------- gate phase ----------------
            with tc.tile_pool(name="gpool", bufs=1) as gp, \
                 tc.tile_pool(name="gpsum", bufs=2, space="PSUM") as gps:
                xTf = gp.tile([128, 8, T], f32, tag="xTf")
                nc.sync.dma_start(xTf[:], xT_d[:])
                wa = gp.tile([128, E, 8, R], f32, tag="wa")
                nc.sync.dma_start(wa[:], WA_d[:])
                esel = gp.tile([128, E, E], f32, tag="esel")
                nc.sync.dma_start(esel[:], esel_d[:])
                bsel = gp.tile([8, E, 128], f32, tag="bsel")
                nc.sync.dma_start(bsel[:], bsel_d[:])
                ident = gp.tile([128, 128], f32, tag="ident")
                nc.sync.dma_start(ident[:], ident_d[:])

                gh2 = gp.tile([128, E, T], f32, tag="gh2")
                nc.vector.memset(gh2[:], 0.0)

                for e in range(E):
                    ghp = gps.tile([64, T], f32, tag="gh")
                    for k in range(8):
                        nc.tensor.matmul(ghp[:], wa[:, e, k, :], xTf[:, k, :],
                                         start=(k == 0), stop=(k == 7))
                    nc.scalar.copy(gh_bf[0:64, e, :], ghp[:])
                    nc.scalar.square(gh2[0:64, e, :], ghp[:])

                # token-major sum of squares: s_tok[t, e] per 128-token chunk
                s_all = gp.tile([128, 4, E], f32, tag="s_all")
                for c in range(4):
                    stp = gps.tile([128, E], f32, tag="stok")
                    for e in range(E):
                        nc.tensor.matmul(stp[:], gh2[:, e, c * 128:(c + 1) * 128],
                                         esel[:, e, :],
                                         start=(e == 0), stop=(e == E - 1))
                    nc.scalar.sqrt(s_all[:, c, :], stp[:])

                # top-2 + softmax over E=8 per token
                m1 = gp.tile([128, 4], f32, tag="m1")
                nc.vector.reduce_max(m1[:], s_all[:], axis=AX.X)
                m1b = m1[:, :, None].to_broadcast((128, 4, E))
                eqm = gp.tile([128, 4, E], f32, tag="eqm")
                nc.vector.tensor_tensor(eqm[:], s_all[:], m1b, OP.is_ge)
                s2 = gp.tile([128, 4, E], f32, tag="s2")
                nc.vector.scalar_tensor_tensor(s2[:], eqm[:], -1e30, s_all[:],
                                               OP.mult, OP.add)
                m2 = gp.tile([128, 4], f32, tag="m2")
                nc.vector.reduce_max(m2[:], s2[:], axis=AX.X)
                m2b = m2[:, :, None].to_broadcast((128, 4, E))

                d1 = gp.tile([128, 4, E], f32, tag="d1")
                nc.vector.tensor_tensor(d1[:], s_all[:], m1b, OP.subtract)
                e1 = gp.tile([128, 4, E], f32, tag="e1")
                nc.scalar.activation(e1[:], d1[:], AF.Exp)
                dm2 = gp.tile([128, 4], f32, tag="dm2")
                nc.vector.tensor_tensor(dm2[:], m2[:], m1[:], OP.subtract)
                em2 = gp.tile([128, 4], f32, tag="em2")
                nc.scalar.activation(em2[:], dm2[:], AF.Exp)
                den = gp.tile([128, 4], f32, tag="den")
                nc.vector.tensor_scalar_add(den[:], em2[:], 1.0)
                rec = gp.tile([128, 4], f32, tag="rec")
                nc.vector.reciprocal(rec[:], den[:])
                recb = rec[:, :, None].to_broadcast((128, 4, E))
                mask2 = gp.tile([128, 4, E], f32, tag="mask2")
                nc.vector.tensor_tensor(mask2[:], s_all[:], m2b, OP.is_ge)
                wm = gp.tile([128, 4, E], f32, tag="wm")
                nc.vector.tensor_tensor(wm[:], e1[:], mask2[:], OP.mult)
                w_all = gp.tile([128, 4, E], f32, tag="w_all")
                nc.vector.tensor_tensor(w_all[:], wm[:], recb, OP.mult)

                # transpose back: w_all [128t, c, e] -> wT [8e, T]
                wT = gp.tile([8, T], f32, tag="wT")
                for c in range(4):
                    wtp = gps.tile([8, 128], f32, tag="wtp")
                    nc.tensor.transpose(wtp[:], w_all[:, c, :], ident[:])
                    nc.vector.tensor_copy(wT[:, c * 128:(c + 1) * 128], wtp[:])

                # broadcast across partitions: w_bc[:, e, t] = wT[e, t]
                for e in range(E):
                    wbp = gps.tile([128, T], f32, tag="wbp")
                    nc.tensor.matmul(wbp[:], bsel[:, e, :], wT[:],
                                     start=True, stop=True)
                    nc.scalar.copy(w_bc[:, e, :], wbp[:])

            # ---------------- main expert loop ----------------
            with tc.tile_pool(name="wpool", bufs=2) as wp, \
                 tc.tile_pool(name="xspool", bufs=2) as xsp, \
                 tc.tile_pool(name="hpool", bufs=2) as hp, \
                 tc.tile_pool(name="spool", bufs=3) as sp, \
                 tc.tile_pool(name="psum_mm", bufs=2, space="PSUM") as pmm, \
                 tc.tile_pool(name="psum_y", bufs=4, space="PSUM") as pyy:

                for e in range(E):
                    x_s = xsp.tile([128, 8, T], bf16, tag="x_s")
                    for k in range(8):
                        nc.vector.tensor_tensor(x_s[:, k, :], xTbf[:, k, :],
                                                w_bc[:, e, :], OP.mult)
                    for fc in range(2):
                        wup_c = wp.tile([128, 8, FCH], bf16, tag="wup")
                        nc.sync.dma_start(
                            wup_c[:], WUP_d[e, :, :, fc * FCH:(fc + 1) * FCH])
                        wdn_c = wp.tile([128, 8, H], bf16, tag="wdn")
                        nc.sync.dma_start(
                            wdn_c[:], WDN_d[e, :, fc * 8:(fc + 1) * 8, :])
                        wb_c = wp.tile([128, FCH], bf16, tag="wb")
                        nc.sync.dma_start(
                            wb_c[:], WB_d[e, :, fc * FCH:(fc + 1) * FCH])

                        for tg in range(2):
                            hbuf = hp.tile([128, 8, TG], bf16, tag="h")
                            for ft in range(8):
                                gpsm = pmm.tile([128, TG], f32, tag="g")
                                nc.tensor.matmul(
                                    gpsm[:],
                                    wb_c[:, ft * 128:(ft + 1) * 128],
                                    gh_bf[:, e, tg * TG:(tg + 1) * TG],
                                    start=True, stop=True)
                                upsm = pmm.tile([128, TG], f32, tag="up")
                                for k in range(8):
                                    nc.tensor.matmul(
                                        upsm[:],
                                        wup_c[:, k, ft * 128:(ft + 1) * 128],
                                        x_s[:, k, tg * TG:(tg + 1) * TG],
                                        start=(k == 0), stop=(k == 7))
                                sil = sp.tile([128, TG], bf16, tag="sil")
                                nc.scalar.activation(sil[:], gpsm[:], AF.Silu)
                                nc.vector.tensor_tensor(hbuf[:, ft, :], sil[:],
                                                        upsm[:], OP.mult)
                            # down: token-major output, contract over f
                            for tc2 in range(2):
                                for hh in range(2):
                                    yp = pyy.tile([128, 512], f32, tag="yp")
                                    for ft in range(8):
                                        nc.tensor.matmul(
                                            yp[:],
                                            hbuf[:, ft, tc2 * 128:(tc2 + 1) * 128],
                                            wdn_c[:, ft, hh * 512:(hh + 1) * 512],
                                            start=(ft == 0), stop=(ft == 7))
                                    ysl = yT[:, tg * 2 + tc2,
                                             hh * 512:(hh + 1) * 512]
                                    nc.vector.tensor_tensor(ysl, ysl, yp[:],
                                                            OP.add)

                nc.sync.dma_start(out_d[:], yT[:])

    nc.compile()
    return nc


def _get_nc():
    if "nc" not in _NC_CACHE:
        _NC_CACHE["nc"] = _build_nc()
    return _NC_CACHE["nc"]


def _prep_inputs(hidden_states, W_A, W_B, W_up, W_down):
    f32 = np.float32
    x2d = np.ascontiguousarray(np.asarray(hidden_states, dtype=f32).reshape(N, H))
    xT = np.ascontiguousarray(x2d.T)                        # [H, N]
    xT_arr = np.ascontiguousarray(
        xT.reshape(8, 128, N).transpose(1, 0, 2))           # [128, 8, N]
    xTbf_arr = xT_arr.astype(BF16)

    W_A = np.asarray(W_A, dtype=f32)
    W_B = np.asarray(W_B, dtype=f32)
    W_up = np.asarray(W_up, dtype=f32)
    W_down = np.asarray(W_down, dtype=f32)

    # WA: [E,R,H] -> [128, E, 8, R] fp32
    WAh = np.ascontiguousarray(
        W_A.transpose(0, 2, 1).reshape(E, 8, 128, R).transpose(2, 0, 1, 3))
    # WB: [E,F,R] -> [E, 128(R pad), F] bf16
    WBt = W_B.transpose(0, 2, 1)                            # [E, R, F]
    WBh = np.zeros((E, 128, F), dtype=BF16)
    WBh[:, :R, :] = WBt.astype(BF16)
    # WUP: [E,F,H] -> [E, 128, 8, F] bf16   (h = k*128 + p)
    WUPh = np.ascontiguousarray(
        W_up.transpose(0, 2, 1).reshape(E, 8, 128, F).transpose(0, 2, 1, 3)
    ).astype(BF16)
    # WDN: [E,H,F] -> [E, 128, 16, H] bf16  (f = k*128 + p)
    WDNh = np.ascontiguousarray(
        W_down.transpose(0, 2, 1).reshape(E, 16, 128, H).transpose(0, 2, 1, 3)
    ).astype(BF16)

    esel = np.zeros((128, E, E), dtype=f32)
    for e in range(E):
        esel[:R, e, e] = 1.0
    bsel = np.zeros((8, E, 128), dtype=f32)
    for e in range(E):
        bsel[e, e, :] = 1.0
    ident = np.eye(128, dtype=f32)

    shared = dict(WA=WAh, WB=WBh, WUP=WUPh, WDN=WDNh,
                  esel=esel, bsel=bsel, ident=ident)
    in_maps = []
    for c in range(NCORES):
        m = dict(shared)
        m["xT"] = np.ascontiguousarray(xT_arr[:, :, c * T:(c + 1) * T])
        m["xTbf"] = np.ascontiguousarray(xTbf_arr[:, :, c * T:(c + 1) * T])
        in_maps.append(m)
    return in_maps


def kernel(hidden_states, W_A, W_B, W_up, W_down):
    global LAST_RESULT
    trace = _maybe_install_trace_hook()
    from concourse import bass_utils

    nc = _get_nc()
    in_maps = _prep_inputs(hidden_states, W_A, W_B, W_up, W_down)
    res = bass_utils.run_bass_kernel_spmd(
        nc, in_maps, core_ids=list(range(NCORES)), trace=trace)
    LAST_RESULT = res

    out = np.empty((N, H), dtype=np.float32)
    for c in range(NCORES):
        arr = res.results[c]["out"]                        # [128, 4, H]
        out[c * T:(c + 1) * T] = arr.transpose(1, 0, 2).reshape(T, H)
    return out.reshape(B, S, H)


# revision 5
# speedup vs baseline: 1.5897x; 1.5897x over previous
# Trainium2 Bass kernel for AoE-style MoE (dense formulation).
#
# Problem: E=8 experts, top-K=2, H=1024, F=2048, low-rank gate R=64,
# tokens N = 2*2048 = 4096.  Reference computes every expert densely with
# zero combine-weight for unselected experts; we do the same, sharding the
# token axis across 8 NeuronCores (data parallel, no collectives).
#
# Per core (T=512 tokens):
#   gate:  gh_e = W_A[e] @ x  (fp32 matmuls, exact top-2 selection)
#          ssq  = sum_r gh^2  -> token-major scores via selector matmul
#          top-2 + softmax on DVE/ACT, weights w[t,e]
#          w broadcast across partitions via selector matmul, folded into x
#   main:  up_e = W_up[e] @ (x*w_e)   (bf16)
#          g_e  = W_B[e] @ gh_e       (bf16)
#          h_e  = silu(g_e) * up_e    (bf16)
#          y   += W_down[e].T-contract h_e   (token-major PSUM, fp32 accum)
#
# kernel(**inputs) takes full unsharded inputs, returns full output.

import os
import sys
import types
import numpy as np
import ml_dtypes

E, TOPK, H, F, R = 8, 2, 1024, 2048, 64
B, S = 2, 2048
N = B * S            # 4096 tokens
NCORES = 8
T = N // NCORES      # 512 tokens per core
TG = 256             # token group for up/g matmuls
FCH = 1024           # F chunk (half of F) streamed per weight DMA

BF16 = ml_dtypes.bfloat16


def _maybe_install_trace_hook():
    """Install the axon NTFF profiling hook if requested and available."""
    if os.environ.get("MOE_TRACE") != "1":
        return False
    try:
        import antenv.axon_hooks  # noqa: F401
        return True
    except ImportError:
        pass
    try:
        if "/root/.axon_site" not in sys.path:
            sys.path.insert(0, "/root/.axon_site")
        from trn_agent_boot.trn_boot import _ntff_profile_via_ctypes
        hook = _ntff_profile_via_ctypes("/opt/axon/libaxon_pjrt.so")
        mod = types.ModuleType("antenv.axon_hooks")
        mod.get_axon_ntff_profile_hook = lambda: hook
        mod.set_axon_ntff_profile_hook = lambda h: None
        sys.modules["antenv.axon_hooks"] = mod
        return True
    except Exception:
        return False


_NC_CACHE = {}
LAST_RESULT = None  # BassKernelResults of the most recent run (for profiling)

C = 256              # per-(core, expert) slot capacity for sparse dispatch


def _build_nc_sparse():
    import concourse.mybir as mybir
    import concourse.tile as tile
    from concourse import bacc

    f32 = mybir.dt.float32
    bf16 = mybir.dt.bfloat16
    AF = mybir.ActivationFunctionType
    OP = mybir.AluOpType
    AX = mybir.AxisListType

    nc = bacc.Bacc("TRN2", target_bir_lowering=False, debug=False,
                   num_devices=NCORES)

    xT_d = nc.dram_tensor("xT", [128, 8, T], f32, kind="ExternalInput")
    xtok_d = nc.dram_tensor("xtok", [128, 4, H], bf16, kind="ExternalInput")
    WApk_d = nc.dram_tensor("WApk", [128, 4, 8, 128], f32, kind="ExternalInput")
    WAbf_d = nc.dram_tensor("WAbf", [128, E, 8, R], bf16, kind="ExternalInput")
    WB_d = nc.dram_tensor("WB", [E, 128, F], bf16, kind="ExternalInput")
    WUP_d = nc.dram_tensor("WUP", [E, 128, 8, F], bf16, kind="ExternalInput")
    WDN_d = nc.dram_tensor("WDN", [E, 128, 16, H], bf16, kind="ExternalInput")
    esel_d = nc.dram_tensor("esel", [128, 4, E], f32, kind="ExternalInput")
    bsel_d = nc.dram_tensor("bsel", [8, E, 128], f32, kind="ExternalInput")
    ident_d = nc.dram_tensor("ident", [128, 128], f32, kind="ExternalInput")
    sbc_d = nc.dram_tensor("sbc", [128, 2], f32, kind="ExternalInput")
    slotbc_d = nc.dram_tensor("slotbc", [128, C], f32, kind="ExternalInput")
    out_d = nc.dram_tensor("out", [128, 4, H], f32, kind="ExternalOutput")

    with tile.TileContext(nc) as tc:
        from contextlib import ExitStack
        with ExitStack() as ctx:
            pp = ctx.enter_context(tc.tile_pool(name="persist", bufs=1))

            xtok = pp.tile([128, 4, H], bf16, tag="xtok")
            nc.sync.dma_start(xtok[:], xtok_d[:])
            wabf = pp.tile([128, E, 8, R], bf16, tag="wabf")
            nc.sync.dma_start(wabf[:], WAbf_d[:])
            ident = pp.tile([128, 128], f32, tag="ident")
            nc.sync.dma_start(ident[:], ident_d[:])
            sbc = pp.tile([128, 2], f32, tag="sbc")
            nc.sync.dma_start(sbc[:], sbc_d[:])
            slotbc = pp.tile([128, C], f32, tag="slotbc")
            nc.sync.dma_start(slotbc[:], slotbc_d[:])
            bsel = pp.tile([8, E, 128], f32, tag="bsel")
            nc.sync.dma_start(bsel[:], bsel_d[:])

            qw = pp.tile([128, 2 * E, T], bf16, tag="qw")       # [slot, e*2+sc, t]
            y_all = pp.tile([128, 2 * E, H], bf16, tag="y_all")  # [slot, e*2+sc, h]
            out_sb = pp.tile([128, 4, H], f32, tag="out_sb")
            pos2_tok = pp.tile([128, 4, E], f32, tag="pos2_tok")
            wT = pp.tile([8, T], f32, tag="wT")

            # ---------------- gate phase (pair-packed fp32) ----------------
            with tc.tile_pool(name="gpool", bufs=1) as gp, \
                 tc.tile_pool(name="gpsum", bufs=2, space="PSUM") as gps, \
                 tc.tile_pool(name="gpsum1", bufs=1, space="PSUM") as gps1:
                xTf = gp.tile([128, 8, T], f32, tag="xTf")
                nc.sync.dma_start(xTf[:], xT_d[:])
                wapk = gp.tile([128, 4, 8, 128], f32, tag="wapk")
                nc.sync.dma_start(wapk[:], WApk_d[:])
                esel = gp.tile([128, 4, E], f32, tag="esel")
                nc.sync.dma_start(esel[:], esel_d[:])

                gh2 = gp.tile([128, 4, T], f32, tag="gh2")
                for pr in range(4):
                    ghp = gps.tile([128, T], f32, tag="gh")
                    for k in range(8):
                        nc.tensor.matmul(ghp[:], wapk[:, pr, k, :], xTf[:, k, :],
                                         start=(k == 0), stop=(k == 7))
                    nc.scalar.square(gh2[:, pr, :], ghp[:])

                s_all = gp.tile([128, 4, E], f32, tag="s_all")
                for c in range(4):
                    stp = gps1.tile([128, E], f32, tag="stok")
                    for pr in range(4):
                        nc.tensor.matmul(stp[:], gh2[:, pr, c * 128:(c + 1) * 128],
                                         esel[:, pr, :],
                                         start=(pr == 0), stop=(pr == 3))
                    nc.scalar.sqrt(s_all[:, c, :], stp[:])

                # top-2 + softmax over E per token
                m1 = gp.tile([128, 4], f32, tag="m1")
                nc.vector.reduce_max(m1[:], s_all[:], axis=AX.X)
                m1b = m1[:, :, None].to_broadcast((128, 4, E))
                eqm = gp.tile([128, 4, E], f32, tag="eqm")
                nc.vector.tensor_tensor(eqm[:], s_all[:], m1b, OP.is_ge)
                s2 = gp.tile([128, 4, E], f32, tag="s2")
                nc.vector.scalar_tensor_tensor(s2[:], eqm[:], -1e30, s_all[:],
                                               OP.mult, OP.add)
                m2 = gp.tile([128, 4], f32, tag="m2")
                nc.vector.reduce_max(m2[:], s2[:], axis=AX.X)
                m2b = m2[:, :, None].to_broadcast((128, 4, E))

                d1 = gp.tile([128, 4, E], f32, tag="d1")
                nc.vector.tensor_tensor(d1[:], s_all[:], m1b, OP.subtract)
                e1 = gp.tile([128, 4, E], f32, tag="e1")
                nc.scalar.activation(e1[:], d1[:], AF.Exp)
                dm2 = gp.tile([128, 4], f32, tag="dm2")
                nc.vector.tensor_tensor(dm2[:], m2[:], m1[:], OP.subtract)
                em2 = gp.tile([128, 4], f32, tag="em2")
                nc.scalar.activation(em2[:], dm2[:], AF.Exp)
                den = gp.tile([128, 4], f32, tag="den")
                nc.vector.tensor_scalar_add(den[:], em2[:], 1.0)
                rec = gp.tile([128, 4], f32, tag="rec")
                nc.vector.reciprocal(rec[:], den[:])
                recb = rec[:, :, None].to_broadcast((128, 4, E))
                mask2 = gp.tile([128, 4, E], f32, tag="mask2")
                nc.vector.tensor_tensor(mask2[:], s_all[:], m2b, OP.is_ge)
                wm = gp.tile([128, 4, E], f32, tag="wm")
                nc.vector.tensor_tensor(wm[:], e1[:], mask2[:], OP.mult)
                w_all = gp.tile([128, 4, E], f32, tag="w_all")
                nc.vector.tensor_tensor(w_all[:], wm[:], recb, OP.mult)

                # transpose: w_all [128t, c, e] -> wT [8e, T]
                for c in range(4):
                    wtp = gps1.tile([8, 128], f32, tag="wtp")
                    nc.tensor.transpose(wtp[:], w_all[:, c, :], ident[:])
                    nc.vector.tensor_copy(wT[:, c * 128:(c + 1) * 128], wtp[:])

                # ---- routing tables ----
                mask = gp.tile([8, T], f32, tag="mask")
                nc.vector.tensor_scalar(mask[:], wT[:], 0.0, None, OP.is_gt)
                zeros8 = gp.tile([8, T], f32, tag="zeros8")
                nc.vector.memset(zeros8[:], 0.0)
                incl = gp.tile([8, T], f32, tag="incl")
                nc.vector.tensor_tensor_scan(incl[:], mask[:], zeros8[:], 0.0,
                                             OP.add, OP.add)
                pos = gp.tile([8, T], f32, tag="pos")
                nc.vector.tensor_tensor(pos[:], incl[:], mask[:], OP.subtract)
                # pos2 = pos where selected else -1e6
                posb = gp.tile([8, T], f32, tag="posb")
                nc.vector.tensor_scalar_add(posb[:], pos[:], 1e6)
                posm = gp.tile([8, T], f32, tag="posm")
                nc.vector.tensor_tensor(posm[:], posb[:], mask[:], OP.mult)
                pos2 = gp.tile([8, T], f32, tag="pos2")
                nc.vector.tensor_scalar_add(pos2[:], posm[:], -1e6)

                # pos2_tok [128t, c, e] via K=8 matmul with I8
                for c in range(4):
                    ptp = gps1.tile([128, E], f32, tag="ptp")
                    nc.tensor.matmul(ptp[:], pos2[:, c * 128:(c + 1) * 128],
                                     ident[0:8, 0:8], start=True, stop=True)
                    nc.vector.tensor_copy(pos2_tok[:, c, :], ptp[:])

                # Qw chunks: [slot(part), t] = (pos2_bc == slot_id) * w_bc
                for e in range(E):
                    wbp = gps1.tile([128, T], f32, tag="wbp")
                    nc.tensor.matmul(wbp[:], bsel[:, e, :], wT[:],
                                     start=True, stop=True)
                    wbs = gp.tile([128, T], f32, tag="wbs")
                    nc.scalar.copy(wbs[:], wbp[:])
                    pbp = gps1.tile([128, T], f32, tag="pbp")
                    nc.tensor.matmul(pbp[:], bsel[:, e, :], pos2[:],
                                     start=True, stop=True)
                    for sc in range(2):
                        nc.vector.scalar_tensor_tensor(
                            qw[:, e * 2 + sc, :], pbp[:], sbc[:, sc:sc + 1],
                            wbs[:], OP.is_equal, OP.mult)

            # ---------------- expert loop (sparse FFN) ----------------
            with tc.tile_pool(name="wpool", bufs=2) as wp, \
                 tc.tile_pool(name="ptpool", bufs=2) as ptp_pool, \
                 tc.tile_pool(name="xgpool", bufs=2) as xgp_pool, \
                 tc.tile_pool(name="hpool", bufs=2) as hp, \
                 tc.tile_pool(name="spool", bufs=3) as sp, \
                 tc.tile_pool(name="ps_xg", bufs=1, space="PSUM") as ps_xg, \
                 tc.tile_pool(name="ps_gh", bufs=1, space="PSUM") as ps_gh, \
                 tc.tile_pool(name="ps_g", bufs=1, space="PSUM") as ps_g, \
                 tc.tile_pool(name="ps_up", bufs=2, space="PSUM") as ps_up, \
                 tc.tile_pool(name="ps_ya", bufs=2, space="PSUM") as ps_ya:

                for e in range(E):
                    # dispatch table PT_e [128t, tc, C] (0/1, unweighted)
                    pt_e = ptp_pool.tile([128, 4, C], bf16, tag="pt")
                    for c in range(4):
                        nc.vector.tensor_scalar(
                            pt_e[:, c, :], slotbc[:], pos2_tok[:, c, e:e + 1],
                            None, OP.is_equal)
                    # gather: x_g [128h, 8, C]
                    x_g = xgp_pool.tile([128, 8, C], bf16, tag="x_g")
                    for hh in range(8):
                        xgp = ps_xg.tile([128, C], f32, tag="xg")
                        for c in range(4):
                            nc.tensor.matmul(
                                xgp[:], xtok[:, c, hh * 128:(hh + 1) * 128],
                                pt_e[:, c, :], start=(c == 0), stop=(c == 3))
                        nc.scalar.copy(x_g[:, hh, :], xgp[:])
                    # recompute gh for gathered tokens (bf16)
                    ghg = xgp_pool.tile([128, C], bf16, tag="ghg")
                    nc.vector.memset(ghg[64:128, :], 0.0)
                    ghp2 = ps_gh.tile([64, C], f32, tag="ghg")
                    for k in range(8):
                        nc.tensor.matmul(ghp2[:], wabf[:, e, k, :], x_g[:, k, :],
                                         start=(k == 0), stop=(k == 7))
                    nc.scalar.copy(ghg[0:64, :], ghp2[:])

                    for fc in range(2):
                        wup_c = wp.tile([128, 8, FCH], bf16, tag="wup")
                        nc.sync.dma_start(
                            wup_c[:], WUP_d[e, :, :, fc * FCH:(fc + 1) * FCH])
                        wdn_c = wp.tile([128, 8, H], bf16, tag="wdn")
                        nc.sync.dma_start(
                            wdn_c[:], WDN_d[e, :, fc * 8:(fc + 1) * 8, :])
                        wb_c = wp.tile([128, FCH], bf16, tag="wb")
                        nc.sync.dma_start(
                            wb_c[:], WB_d[e, :, fc * FCH:(fc + 1) * FCH])

                        hbuf = hp.tile([128, 8, C], bf16, tag="h")
                        for ft in range(8):
                            gpsm = ps_g.tile([128, C], f32, tag="g")
                            nc.tensor.matmul(gpsm[:],
                                             wb_c[:, ft * 128:(ft + 1) * 128],
                                             ghg[:], start=True, stop=True)
                            upsm = ps_up.tile([128, C], f32, tag="up")
                            for k in range(8):
                                nc.tensor.matmul(
                                    upsm[:], wup_c[:, k, ft * 128:(ft + 1) * 128],
                                    x_g[:, k, :], start=(k == 0), stop=(k == 7))
                            sil = sp.tile([128, C], bf16, tag="sil")
                            nc.scalar.activation(sil[:], gpsm[:], AF.Silu)
                            nc.vector.tensor_tensor(hbuf[:, ft, :], sil[:],
                                                    upsm[:], OP.mult)
                        for sc in range(2):
                            for hh in range(2):
                                ya = ps_ya.tile([128, 512], f32, tag="ya")
                                for ft in range(8):
                                    nc.tensor.matmul(
                                        ya[:],
                                        hbuf[:, ft, sc * 128:(sc + 1) * 128],
                                        wdn_c[:, ft, hh * 512:(hh + 1) * 512],
                                        start=(ft == 0), stop=(ft == 7))
                                ysl = y_all[:, e * 2 + sc, hh * 512:(hh + 1) * 512]
                                if fc == 0:
                                    nc.scalar.copy(ysl, ya[:])
                                else:
                                    nc.vector.tensor_tensor(ysl, ysl, ya[:],
                                                            OP.add)

            # ---------------- combine ----------------
            with tc.tile_pool(name="ps_c", bufs=4, space="PSUM") as ps_c:
                for c in range(4):
                    for hh in range(2):
                        cp = ps_c.tile([128, 512], f32, tag="cp")
                        for idx in range(2 * E):
                            nc.tensor.matmul(
                                cp[:], qw[:, idx, c * 128:(c + 1) * 128],
                                y_all[:, idx, hh * 512:(hh + 1) * 512],
                                start=(idx == 0), stop=(idx == 2 * E - 1))
                        nc.scalar.copy(out_sb[:, c, hh * 512:(hh + 1) * 512], cp[:])
                nc.sync.dma_start(out_d[:], out_sb[:])

    nc.compile()
    return nc


def _build_nc():
    import concourse.mybir as mybir
    import concourse.tile as tile
    from concourse import bacc

    f32 = mybir.dt.float32
    bf16 = mybir.dt.bfloat16
    AF = mybir.ActivationFunctionType
    OP = mybir.AluOpType
    AX = mybir.AxisListType

    nc = bacc.Bacc("TRN2", target_bir_lowering=False, debug=False,
                   num_devices=NCORES)

    xT_d = nc.dram_tensor("xT", [128, 8, T], f32, kind="ExternalInput")
    xTbf_d = nc.dram_tensor("xTbf", [128, 8, T], bf16, kind="ExternalInput")
    WA_d = nc.dram_tensor("WA", [128, E, 8, R], f32, kind="ExternalInput")
    WB_d = nc.dram_tensor("WB", [E, 128, F], bf16, kind="ExternalInput")
    WUP_d = nc.dram_tensor("WUP", [E, 128, 8, F], bf16, kind="ExternalInput")
    WDN_d = nc.dram_tensor("WDN", [E, 128, 16, H], bf16, kind="ExternalInput")
    esel_d = nc.dram_tensor("esel", [128, E, E], f32, kind="ExternalInput")
    bsel_d = nc.dram_tensor("bsel", [8, E, 128], f32, kind="ExternalInput")
    ident_d = nc.dram_tensor("ident", [128, 128], f32, kind="ExternalInput")
    out_d = nc.dram_tensor("out", [128, 4, H], f32, kind="ExternalOutput")

    with tile.TileContext(nc) as tc:
        from contextlib import ExitStack
        with ExitStack() as ctx:
            pp = ctx.enter_context(tc.tile_pool(name="persist", bufs=1))

            # persistent SBUF tensors
            xTbf = pp.tile([128, 8, T], bf16, tag="xTbf")
            nc.sync.dma_start(xTbf[:], xTbf_d[:])
            gh_bf = pp.tile([128, E, T], bf16, tag="gh_bf")
            nc.vector.memset(gh_bf[:], 0.0)
            w_bc = pp.tile([128, E, T], bf16, tag="w_bc")
            yT = pp.tile([128, 4, H], f32, tag="yT")
            nc.vector.memset(yT[:], 0.0)

            # ---------------- gate phase ----------------
            with tc.tile_pool(name="gpool", bufs=1) as gp, \
                 tc.tile_pool(name="gpsum", bufs=2, space="PSUM") as gps:
                xTf = gp.tile([128, 8, T], f32, tag="xTf")
                nc.sync.dma_start(xTf[:], xT_d[:])
                wa = gp.tile([128, E, 8, R], f32, tag="wa")
                nc.sync.dma_start(wa[:], WA_d[:])
                esel = gp.tile([128, E, E], f32, tag="esel")
                nc.sync.dma_start(esel[:], esel_d[:])
                bsel = gp.tile([8, E, 128], f32, tag="bsel")
                nc.sync.dma_start(bsel[:], bsel_d[:])
                ident = gp.tile([128, 128], f32, tag="ident")
                nc.sync.dma_start(ident[:], ident_d[:])

                gh2 = gp.tile([128, E, T], f32, tag="gh2")
                nc.vector.memset(gh2[:], 0.0)

                for e in range(E):
                    ghp = gps.tile([64, T], f32, tag="gh")
                    for k in range(8):
                        nc.tensor.matmul(ghp[:], wa[:, e, k, :], xTf[:, k, :],
                                         start=(k == 0), stop=(k == 7))
                    nc.scalar.copy(gh_bf[0:64, e, :], ghp[:])
                    nc.scalar.square(gh2[0:64, e, :], ghp[:])

                # token-major sum of squares: s_tok[t, e] per 128-token chunk
                s_all = gp.tile([128, 4, E], f32, tag="s_all")
                for c in range(4):
                    stp = gps1.tile([128, E], f32, tag="stok")
                    for e in range(E):
                        nc.tensor.matmul(stp[:], gh2[:, e, c * 128:(c + 1) * 128],
                                         esel[:, e, :],
                                         start=(e == 0), stop=(e == E - 1))
                    nc.scalar.sqrt(s_all[:, c, :], stp[:])

                # top-2 + softmax over E=8 per token
                m1 = gp.tile([128, 4], f32, tag="m1")
                nc.vector.reduce_max(m1[:], s_all[:], axis=AX.X)
                m1b = m1[:, :, None].to_broadcast((128, 4, E))
                eqm = gp.tile([128, 4, E], f32, tag="eqm")
                nc.vector.tensor_tensor(eqm[:], s_all[:], m1b, OP.is_ge)
                s2 = gp.tile([128, 4, E], f32, tag="s2")
                nc.vector.scalar_tensor_tensor(s2[:], eqm[:], -1e30, s_all[:],
                                               OP.mult, OP.add)
                m2 = gp.tile([128, 4], f32, tag="m2")
                nc.vector.reduce_max(m2[:], s2[:], axis=AX.X)
                m2b = m2[:, :, None].to_broadcast((128, 4, E))

                d1 = gp.tile([128, 4, E], f32, tag="d1")
                nc.vector.tensor_tensor(d1[:], s_all[:], m1b, OP.subtract)
                e1 = gp.tile([128, 4, E], f32, tag="e1")
                nc.scalar.activation(e1[:], d1[:], AF.Exp)
                dm2 = gp.tile([128, 4], f32, tag="dm2")
                nc.vector.tensor_tensor(dm2[:], m2[:], m1[:], OP.subtract)
                em2 = gp.tile([128, 4], f32, tag="em2")
                nc.scalar.activation(em2[:], dm2[:], AF.Exp)
                den = gp.tile([128, 4], f32, tag="den")
                nc.vector.tensor_scalar_add(den[:], em2[:], 1.0)
                rec = gp.tile([128, 4], f32, tag="rec")
                nc.vector.reciprocal(rec[:], den[:])
                recb = rec[:, :, None].to_broadcast((128, 4, E))
                mask2 = gp.tile([128, 4, E], f32, tag="mask2")
                nc.vector.tensor_tensor(mask2[:], s_all[:], m2b, OP.is_ge)
                wm = gp.tile([128, 4, E], f32, tag="wm")
                nc.vector.tensor_tensor(wm[:], e1[:], mask2[:], OP.mult)
                w_all = gp.tile([128, 4, E], f32, tag="w_all")
                nc.vector.tensor_tensor(w_all[:], wm[:], recb, OP.mult)

                # transpose back: w_all [128t, c, e] -> wT [8e, T]
                wT = gp.tile([8, T], f32, tag="wT")
                for c in range(4):
                    wtp = gps1.tile([8, 128], f32, tag="wtp")
                    nc.tensor.transpose(wtp[:], w_all[:, c, :], ident[:])
                    nc.vector.tensor_copy(wT[:, c * 128:(c + 1) * 128], wtp[:])

                # broadcast across partitions: w_bc[:, e, t] = wT[e, t]
                for e in range(E):
                    wbp = gps1.tile([128, T], f32, tag="wbp")
                    nc.tensor.matmul(wbp[:], bsel[:, e, :], wT[:],
                                     start=True, stop=True)
                    nc.scalar.copy(w_bc[:, e, :], wbp[:])

            # ---------------- main expert loop ----------------
            with tc.tile_pool(name="wpool", bufs=2) as wp, \
                 tc.tile_pool(name="xspool", bufs=2) as xsp, \
                 tc.tile_pool(name="hpool", bufs=2) as hp, \
                 tc.tile_pool(name="spool", bufs=3) as sp, \
                 tc.tile_pool(name="psum_mm", bufs=2, space="PSUM") as pmm, \
                 tc.tile_pool(name="psum_y", bufs=4, space="PSUM") as pyy:

                for e in range(E):
                    x_s = xsp.tile([128, 8, T], bf16, tag="x_s")
                    for k in range(8):
                        nc.vector.tensor_tensor(x_s[:, k, :], xTbf[:, k, :],
                                                w_bc[:, e, :], OP.mult)
                    for fc in range(2):
                        wup_c = wp.tile([128, 8, FCH], bf16, tag="wup")
                        nc.sync.dma_start(
                            wup_c[:], WUP_d[e, :, :, fc * FCH:(fc + 1) * FCH])
                        wdn_c = wp.tile([128, 8, H], bf16, tag="wdn")
                        nc.sync.dma_start(
                            wdn_c[:], WDN_d[e, :, fc * 8:(fc + 1) * 8, :])
                        wb_c = wp.tile([128, FCH], bf16, tag="wb")
                        nc.sync.dma_start(
                            wb_c[:], WB_d[e, :, fc * FCH:(fc + 1) * FCH])

                        for tg in range(2):
                            hbuf = hp.tile([128, 8, TG], bf16, tag="h")
                            for ft in range(8):
                                gpsm = pmm.tile([128, TG], f32, tag="g")
                                nc.tensor.matmul(
                                    gpsm[:],
                                    wb_c[:, ft * 128:(ft + 1) * 128],
                                    gh_bf[:, e, tg * TG:(tg + 1) * TG],
                                    start=True, stop=True)
                                upsm = pmm.tile([128, TG], f32, tag="up")
                                for k in range(8):
                                    nc.tensor.matmul(
                                        upsm[:],
                                        wup_c[:, k, ft * 128:(ft + 1) * 128],
                                        x_s[:, k, tg * TG:(tg + 1) * TG],
                                        start=(k == 0), stop=(k == 7))
                                sil = sp.tile([128, TG], bf16, tag="sil")
                                nc.scalar.activation(sil[:], gpsm[:], AF.Silu)
                                nc.vector.tensor_tensor(hbuf[:, ft, :], sil[:],
                                                        upsm[:], OP.mult)
                            # down: token-major output, contract over f
                            for tc2 in range(2):
                                for hh in range(2):
                                    yp = pyy.tile([128, 512], f32, tag="yp")
                                    for ft in range(8):
                                        nc.tensor.matmul(
                                            yp[:],
                                            hbuf[:, ft, tc2 * 128:(tc2 + 1) * 128],
                                            wdn_c[:, ft, hh * 512:(hh + 1) * 512],
                                            start=(ft == 0), stop=(ft == 7))
                                    ysl = yT[:, tg * 2 + tc2,
                                             hh * 512:(hh + 1) * 512]
                                    nc.vector.tensor_tensor(ysl, ysl, yp[:],
                                                            OP.add)

                nc.sync.dma_start(out_d[:], yT[:])

    nc.compile()
    return nc


def _get_nc(impl):
    key = "nc_" + impl
    if key not in _NC_CACHE:
        _NC_CACHE[key] = (_build_nc_sparse() if impl == "sparse" else _build_nc())
    return _NC_CACHE[key]


def _prep_inputs(hidden_states, W_A, W_B, W_up, W_down):
    f32 = np.float32
    x2d = np.ascontiguousarray(np.asarray(hidden_states, dtype=f32).reshape(N, H))
    xT = np.ascontiguousarray(x2d.T)                        # [H, N]
    xT_arr = np.ascontiguousarray(
        xT.reshape(8, 128, N).transpose(1, 0, 2))           # [128, 8, N]
    xTbf_arr = xT_arr.astype(BF16)

    W_A = np.asarray(W_A, dtype=f32)
    W_B = np.asarray(W_B, dtype=f32)
    W_up = np.asarray(W_up, dtype=f32)
    W_down = np.asarray(W_down, dtype=f32)

    # WA: [E,R,H] -> [128, E, 8, R] fp32
    WAh = np.ascontiguousarray(
        W_A.transpose(0, 2, 1).reshape(E, 8, 128, R).transpose(2, 0, 1, 3))
    # WB: [E,F,R] -> [E, 128(R pad), F] bf16
    WBt = W_B.transpose(0, 2, 1)                            # [E, R, F]
    WBh = np.zeros((E, 128, F), dtype=BF16)
    WBh[:, :R, :] = WBt.astype(BF16)
    # WUP: [E,F,H] -> [E, 128, 8, F] bf16   (h = k*128 + p)
    WUPh = np.ascontiguousarray(
        W_up.transpose(0, 2, 1).reshape(E, 8, 128, F).transpose(0, 2, 1, 3)
    ).astype(BF16)
    # WDN: [E,H,F] -> [E, 128, 16, H] bf16  (f = k*128 + p)
    WDNh = np.ascontiguousarray(
        W_down.transpose(0, 2, 1).reshape(E, 16, 128, H).transpose(0, 2, 1, 3)
    ).astype(BF16)

    esel = np.zeros((128, E, E), dtype=f32)
    for e in range(E):
        esel[:R, e, e] = 1.0
    bsel = np.zeros((8, E, 128), dtype=f32)
    for e in range(E):
        bsel[e, e, :] = 1.0
    ident = np.eye(128, dtype=f32)

    shared = dict(WA=WAh, WB=WBh, WUP=WUPh, WDN=WDNh,
                  esel=esel, bsel=bsel, ident=ident)
    in_maps = []
    for c in range(NCORES):
        m = dict(shared)
        m["xT"] = np.ascontiguousarray(xT_arr[:, :, c * T:(c + 1) * T])
        m["xTbf"] = np.ascontiguousarray(xTbf_arr[:, :, c * T:(c + 1) * T])
        in_maps.append(m)
    return in_maps


def _prep_inputs_sparse(hidden_states, W_A, W_B, W_up, W_down):
    f32 = np.float32
    x2d = np.ascontiguousarray(np.asarray(hidden_states, dtype=f32).reshape(N, H))
    xT = np.ascontiguousarray(x2d.T)                        # [H, N]
    xT_arr = np.ascontiguousarray(
        xT.reshape(8, 128, N).transpose(1, 0, 2))           # [128, 8, N]
    # token-major x: [128(t%128... t = c*128+p within core), 4, H]
    xtok_arr = np.ascontiguousarray(
        x2d.reshape(NCORES, 4, 128, H).transpose(0, 2, 1, 3)).astype(BF16)

    W_A = np.asarray(W_A, dtype=f32)
    W_B = np.asarray(W_B, dtype=f32)
    W_up = np.asarray(W_up, dtype=f32)
    W_down = np.asarray(W_down, dtype=f32)

    WA_t = W_A.transpose(0, 2, 1).reshape(E, 8, 128, R)     # [E, k, p, R]
    # pair-packed fp32 gate weights: [128, 4, 8, 128] (cols 0:64 even, 64:128 odd)
    WApk = np.zeros((128, 4, 8, 128), dtype=f32)
    for pr in range(4):
        WApk[:, pr, :, 0:64] = WA_t[2 * pr].transpose(1, 0, 2)
        WApk[:, pr, :, 64:128] = WA_t[2 * pr + 1].transpose(1, 0, 2)
    # bf16 gate weights for gathered recompute: [128, E, 8, R]
    WAbf = np.ascontiguousarray(WA_t.transpose(2, 0, 1, 3)).astype(BF16)

    WBt = W_B.transpose(0, 2, 1)                            # [E, R, F]
    WBh = np.zeros((E, 128, F), dtype=BF16)
    WBh[:, :R, :] = WBt.astype(BF16)
    WUPh = np.ascontiguousarray(
        W_up.transpose(0, 2, 1).reshape(E, 8, 128, F).transpose(0, 2, 1, 3)
    ).astype(BF16)
    WDNh = np.ascontiguousarray(
        W_down.transpose(0, 2, 1).reshape(E, 16, 128, H).transpose(0, 2, 1, 3)
    ).astype(BF16)

    esel = np.zeros((128, 4, E), dtype=f32)
    for pr in range(4):
        esel[0:64, pr, 2 * pr] = 1.0
        esel[64:128, pr, 2 * pr + 1] = 1.0
    bsel = np.zeros((8, E, 128), dtype=f32)
    for e in range(E):
        bsel[e, e, :] = 1.0
    ident = np.eye(128, dtype=f32)
    sbc = np.zeros((128, 2), dtype=f32)
    sbc[:, 0] = np.arange(128)
    sbc[:, 1] = np.arange(128) + 128
    slotbc = np.tile(np.arange(C, dtype=f32)[None, :], (128, 1))

    shared = dict(WApk=WApk, WAbf=WAbf, WB=WBh, WUP=WUPh, WDN=WDNh,
                  esel=esel, bsel=bsel, ident=ident, sbc=sbc, slotbc=slotbc)
    in_maps = []
    for c in range(NCORES):
        m = dict(shared)
        m["xT"] = np.ascontiguousarray(xT_arr[:, :, c * T:(c + 1) * T])
        m["xtok"] = np.ascontiguousarray(xtok_arr[c])
        in_maps.append(m)
    return in_maps


def kernel(hidden_states, W_A, W_B, W_up, W_down):
    global LAST_RESULT
    trace = _maybe_install_trace_hook()
    from concourse import bass_utils

    impl = os.environ.get("MOE_IMPL", "sparse")
    nc = _get_nc(impl)
    if impl == "sparse":
        in_maps = _prep_inputs_sparse(hidden_states, W_A, W_B, W_up, W_down)
    else:
        in_maps = _prep_inputs(hidden_states, W_A, W_B, W_up, W_down)
    res = bass_utils.run_bass_kernel_spmd(
        nc, in_maps, core_ids=list(range(NCORES)), trace=trace)
    LAST_RESULT = res

    out = np.empty((N, H), dtype=np.float32)
    for c in range(NCORES):
        arr = res.results[c]["out"]                        # [128, 4, H]
        out[c * T:(c + 1) * T] = arr.transpose(1, 0, 2).reshape(T, H)
    return out.reshape(B, S, H)


# revision 6
# speedup vs baseline: 1.7352x; 1.0915x over previous
# Trainium2 Bass kernel for AoE-style MoE (dense formulation).
#
# Problem: E=8 experts, top-K=2, H=1024, F=2048, low-rank gate R=64,
# tokens N = 2*2048 = 4096.  Reference computes every expert densely with
# zero combine-weight for unselected experts; we do the same, sharding the
# token axis across 8 NeuronCores (data parallel, no collectives).
#
# Per core (T=512 tokens):
#   gate:  gh_e = W_A[e] @ x  (fp32 matmuls, exact top-2 selection)
#          ssq  = sum_r gh^2  -> token-major scores via selector matmul
#          top-2 + softmax on DVE/ACT, weights w[t,e]
#          w broadcast across partitions via selector matmul, folded into x
#   main:  up_e = W_up[e] @ (x*w_e)   (bf16)
#          g_e  = W_B[e] @ gh_e       (bf16)
#          h_e  = silu(g_e) * up_e    (bf16)
#          y   += W_down[e].T-contract h_e   (token-major PSUM, fp32 accum)
#
# kernel(**inputs) takes full unsharded inputs, returns full output.

import os
import sys
import types
import numpy as np
import ml_dtypes

E, TOPK, H, F, R = 8, 2, 1024, 2048, 64
B, S = 2, 2048
N = B * S            # 4096 tokens
NCORES = 8
T = N // NCORES      # 512 tokens per core
TG = 256             # token group for up/g matmuls
FCH = 1024           # F chunk (half of F) streamed per weight DMA

BF16 = ml_dtypes.bfloat16


def _maybe_install_trace_hook():
    """Install the axon NTFF profiling hook if requested and available."""
    if os.environ.get("MOE_TRACE") != "1":
        return False
    try:
        import antenv.axon_hooks  # noqa: F401
        return True
    except ImportError:
        pass
    try:
        if "/root/.axon_site" not in sys.path:
            sys.path.insert(0, "/root/.axon_site")
        from trn_agent_boot.trn_boot import _ntff_profile_via_ctypes
        hook = _ntff_profile_via_ctypes("/opt/axon/libaxon_pjrt.so")
        mod = types.ModuleType("antenv.axon_hooks")
        mod.get_axon_ntff_profile_hook = lambda: hook
        mod.set_axon_ntff_profile_hook = lambda h: None
        sys.modules["antenv.axon_hooks"] = mod
        return True
    except Exception:
        return False


_NC_CACHE = {}
LAST_RESULT = None  # BassKernelResults of the most recent run (for profiling)

C = 256              # per-(core, expert) slot capacity for sparse dispatch


def _build_nc_sparse():
    import concourse.mybir as mybir
    import concourse.tile as tile
    from concourse import bacc

    f32 = mybir.dt.float32
    bf16 = mybir.dt.bfloat16
    AF = mybir.ActivationFunctionType
    OP = mybir.AluOpType
    AX = mybir.AxisListType

    nc = bacc.Bacc("TRN2", target_bir_lowering=False, debug=False,
                   num_devices=NCORES)

    xT_d = nc.dram_tensor("xT", [128, 8, T], f32, kind="ExternalInput")
    xtok_d = nc.dram_tensor("xtok", [128, 4, H], bf16, kind="ExternalInput")
    WApk_d = nc.dram_tensor("WApk", [128, 4, 8, 128], f32, kind="ExternalInput")
    WAbf_d = nc.dram_tensor("WAbf", [128, E, 8, R], bf16, kind="ExternalInput")
    WB_d = nc.dram_tensor("WB", [E, 128, F], bf16, kind="ExternalInput")
    WUP_d = nc.dram_tensor("WUP", [E, 128, 8, F], bf16, kind="ExternalInput")
    WDN_d = nc.dram_tensor("WDN", [E, 128, 16, H], bf16, kind="ExternalInput")
    esel_d = nc.dram_tensor("esel", [128, 4, E], f32, kind="ExternalInput")
    bsel_d = nc.dram_tensor("bsel", [8, E, 128], f32, kind="ExternalInput")
    ident_d = nc.dram_tensor("ident", [128, 128], f32, kind="ExternalInput")
    sbc_d = nc.dram_tensor("sbc", [128, 2], f32, kind="ExternalInput")
    slotbc_d = nc.dram_tensor("slotbc", [128, C], f32, kind="ExternalInput")
    out_d = nc.dram_tensor("out", [128, 4, H], f32, kind="ExternalOutput")

    with tile.TileContext(nc) as tc:
        from contextlib import ExitStack
        with ExitStack() as ctx:
            pp = ctx.enter_context(tc.tile_pool(name="persist", bufs=1))

            xtok = pp.tile([128, 4, H], bf16, tag="xtok")
            wabf = pp.tile([128, E, 8, R], bf16, tag="wabf")
            ident = pp.tile([128, 128], f32, tag="ident")
            nc.sync.dma_start(ident[:], ident_d[:])
            sbc = pp.tile([128, 2], f32, tag="sbc")
            nc.sync.dma_start(sbc[:], sbc_d[:])
            slotbc = pp.tile([128, C], f32, tag="slotbc")
            nc.sync.dma_start(slotbc[:], slotbc_d[:])
            bsel = pp.tile([8, E, 128], f32, tag="bsel")
            nc.sync.dma_start(bsel[:], bsel_d[:])

            qw = pp.tile([128, 2 * E, T], bf16, tag="qw")       # [slot, e*2+sc, t]
            y_all = pp.tile([128, 2 * E, H], bf16, tag="y_all")  # [slot, e*2+sc, h]
            out_sb = pp.tile([128, 4, H], f32, tag="out_sb")
            pos2_tok = pp.tile([128, 4, E], f32, tag="pos2_tok")
            wT = pp.tile([8, T], f32, tag="wT")

            # ---------------- gate phase (pair-packed fp32) ----------------
            with tc.tile_pool(name="gpool", bufs=1) as gp, \
                 tc.tile_pool(name="gpsum", bufs=2, space="PSUM") as gps, \
                 tc.tile_pool(name="gpsum1", bufs=1, space="PSUM") as gps1:
                xTf = gp.tile([128, 8, T], f32, tag="xTf")
                wapk = gp.tile([128, 4, 8, 128], f32, tag="wapk")
                for k in range(8):
                    nc.sync.dma_start(wapk[:, :, k, :], WApk_d[:, :, k, :])
                    nc.sync.dma_start(xTf[:, k, :], xT_d[:, k, :])
                esel = gp.tile([128, 4, E], f32, tag="esel")
                nc.sync.dma_start(esel[:], esel_d[:])
                # expert-loop inputs: queue behind the gate-critical loads
                nc.sync.dma_start(xtok[:], xtok_d[:])
                nc.sync.dma_start(wabf[:], WAbf_d[:])

                gh2 = gp.tile([128, 4, T], f32, tag="gh2")
                for pr in range(4):
                    ghp = gps1.tile([128, T], f32, tag="gh")
                    for k in range(8):
                        nc.tensor.matmul(ghp[:], wapk[:, pr, k, :], xTf[:, k, :],
                                         start=(k == 0), stop=(k == 7))
                    nc.scalar.square(gh2[:, pr, :], ghp[:])

                s_all = gp.tile([128, 4, E], f32, tag="s_all")
                for c in range(4):
                    stp = gps1.tile([128, E], f32, tag="stok")
                    for pr in range(4):
                        nc.tensor.matmul(stp[:], gh2[:, pr, c * 128:(c + 1) * 128],
                                         esel[:, pr, :],
                                         start=(pr == 0), stop=(pr == 3))
                    nc.scalar.sqrt(s_all[:, c, :], stp[:])

                # top-2 + softmax over E per token
                m1 = gp.tile([128, 4], f32, tag="m1")
                nc.vector.reduce_max(m1[:], s_all[:], axis=AX.X)
                m1b = m1[:, :, None].to_broadcast((128, 4, E))
                eqm = gp.tile([128, 4, E], f32, tag="eqm")
                nc.vector.tensor_tensor(eqm[:], s_all[:], m1b, OP.is_ge)
                s2 = gp.tile([128, 4, E], f32, tag="s2")
                nc.vector.scalar_tensor_tensor(s2[:], eqm[:], -1e30, s_all[:],
                                               OP.mult, OP.add)
                m2 = gp.tile([128, 4], f32, tag="m2")
                nc.vector.reduce_max(m2[:], s2[:], axis=AX.X)
                m2b = m2[:, :, None].to_broadcast((128, 4, E))

                d1 = gp.tile([128, 4, E], f32, tag="d1")
                nc.vector.tensor_tensor(d1[:], s_all[:], m1b, OP.subtract)
                e1 = gp.tile([128, 4, E], f32, tag="e1")
                nc.scalar.activation(e1[:], d1[:], AF.Exp)
                dm2 = gp.tile([128, 4], f32, tag="dm2")
                nc.vector.tensor_tensor(dm2[:], m2[:], m1[:], OP.subtract)
                em2 = gp.tile([128, 4], f32, tag="em2")
                nc.scalar.activation(em2[:], dm2[:], AF.Exp)
                den = gp.tile([128, 4], f32, tag="den")
                nc.vector.tensor_scalar_add(den[:], em2[:], 1.0)
                rec = gp.tile([128, 4], f32, tag="rec")
                nc.vector.reciprocal(rec[:], den[:])
                recb = rec[:, :, None].to_broadcast((128, 4, E))
                mask2 = gp.tile([128, 4, E], f32, tag="mask2")
                nc.vector.tensor_tensor(mask2[:], s_all[:], m2b, OP.is_ge)
                wm = gp.tile([128, 4, E], f32, tag="wm")
                nc.vector.tensor_tensor(wm[:], e1[:], mask2[:], OP.mult)
                w_all = gp.tile([128, 4, E], f32, tag="w_all")
                nc.vector.tensor_tensor(w_all[:], wm[:], recb, OP.mult)

                # transpose: w_all [128t, c, e] -> wT [8e, T]
                for c in range(4):
                    wtp = gps1.tile([8, 128], f32, tag="wtp")
                    nc.tensor.transpose(wtp[:], w_all[:, c, :], ident[:])
                    nc.vector.tensor_copy(wT[:, c * 128:(c + 1) * 128], wtp[:])

                # ---- routing tables ----
                mask = gp.tile([8, T], f32, tag="mask")
                nc.vector.tensor_scalar(mask[:], wT[:], 0.0, None, OP.is_gt)
                zeros8 = gp.tile([8, T], f32, tag="zeros8")
                nc.vector.memset(zeros8[:], 0.0)
                incl = gp.tile([8, T], f32, tag="incl")
                nc.vector.tensor_tensor_scan(incl[:], mask[:], zeros8[:], 0.0,
                                             OP.add, OP.add)
                pos = gp.tile([8, T], f32, tag="pos")
                nc.vector.tensor_tensor(pos[:], incl[:], mask[:], OP.subtract)
                # pos2 = pos where selected else -1e6
                posb = gp.tile([8, T], f32, tag="posb")
                nc.vector.tensor_scalar_add(posb[:], pos[:], 1e6)
                posm = gp.tile([8, T], f32, tag="posm")
                nc.vector.tensor_tensor(posm[:], posb[:], mask[:], OP.mult)
                pos2 = gp.tile([8, T], f32, tag="pos2")
                nc.vector.tensor_scalar_add(pos2[:], posm[:], -1e6)

                # pos2_tok [128t, c, e] via K=8 matmul with I8
                for c in range(4):
                    ptp = gps1.tile([128, E], f32, tag="ptp")
                    nc.tensor.matmul(ptp[:], pos2[:, c * 128:(c + 1) * 128],
                                     ident[0:8, 0:8], start=True, stop=True)
                    nc.vector.tensor_copy(pos2_tok[:, c, :], ptp[:])

                # Qw chunks: [slot(part), t] = (pos2_bc == slot_id) * w_bc
                for e in range(E):
                    wbp = gps.tile([128, T], f32, tag="wbp")
                    nc.tensor.matmul(wbp[:], bsel[:, e, :], wT[:],
                                     start=True, stop=True)
                    wbs = gp.tile([128, T], f32, tag="wbs")
                    nc.scalar.copy(wbs[:], wbp[:])
                    pbp = gps.tile([128, T], f32, tag="pbp")
                    nc.tensor.matmul(pbp[:], bsel[:, e, :], pos2[:],
                                     start=True, stop=True)
                    for sc in range(2):
                        nc.vector.scalar_tensor_tensor(
                            qw[:, e * 2 + sc, :], pbp[:], sbc[:, sc:sc + 1],
                            wbs[:], OP.is_equal, OP.mult)

            # ---------------- expert loop (sparse FFN) ----------------
            with tc.tile_pool(name="wpool", bufs=2) as wp, \
                 tc.tile_pool(name="ptpool", bufs=2) as ptp_pool, \
                 tc.tile_pool(name="xgpool", bufs=2) as xgp_pool, \
                 tc.tile_pool(name="hpool", bufs=2) as hp, \
                 tc.tile_pool(name="spool", bufs=3) as sp, \
                 tc.tile_pool(name="ps_xg", bufs=2, space="PSUM") as ps_xg, \
                 tc.tile_pool(name="ps_gh", bufs=1, space="PSUM") as ps_gh, \
                 tc.tile_pool(name="ps_g", bufs=1, space="PSUM") as ps_g, \
                 tc.tile_pool(name="ps_up", bufs=2, space="PSUM") as ps_up, \
                 tc.tile_pool(name="ps_ya", bufs=2, space="PSUM") as ps_ya:

                for e in range(E):
                    # dispatch table PT_e [128t, tc, C] (0/1, unweighted)
                    pt_e = ptp_pool.tile([128, 4, C], bf16, tag="pt")
                    for c in range(4):
                        nc.vector.tensor_scalar(
                            pt_e[:, c, :], slotbc[:], pos2_tok[:, c, e:e + 1],
                            None, OP.is_equal)
                    # gather: x_g [128h, 8, C]
                    x_g = xgp_pool.tile([128, 8, C], bf16, tag="x_g")
                    for hh in range(8):
                        xgp = ps_xg.tile([128, C], f32, tag="xg")
                        for c in range(4):
                            nc.tensor.matmul(
                                xgp[:], xtok[:, c, hh * 128:(hh + 1) * 128],
                                pt_e[:, c, :], start=(c == 0), stop=(c == 3))
                        nc.scalar.copy(x_g[:, hh, :], xgp[:])
                    # recompute gh for gathered tokens (bf16)
                    ghg = xgp_pool.tile([128, C], bf16, tag="ghg")
                    nc.vector.memset(ghg[64:128, :], 0.0)
                    ghp2 = ps_gh.tile([64, C], f32, tag="ghg")
                    for k in range(8):
                        nc.tensor.matmul(ghp2[:], wabf[:, e, k, :], x_g[:, k, :],
                                         start=(k == 0), stop=(k == 7))
                    nc.scalar.copy(ghg[0:64, :], ghp2[:])

                    for fc in range(2):
                        wup_c = wp.tile([128, 8, FCH], bf16, tag="wup")
                        nc.sync.dma_start(
                            wup_c[:], WUP_d[e, :, :, fc * FCH:(fc + 1) * FCH])
                        wdn_c = wp.tile([128, 8, H], bf16, tag="wdn")
                        nc.sync.dma_start(
                            wdn_c[:], WDN_d[e, :, fc * 8:(fc + 1) * 8, :])
                        wb_c = wp.tile([128, FCH], bf16, tag="wb")
                        nc.sync.dma_start(
                            wb_c[:], WB_d[e, :, fc * FCH:(fc + 1) * FCH])

                        hbuf = hp.tile([128, 8, C], bf16, tag="h")
                        for ft in range(8):
                            gpsm = ps_g.tile([128, C], f32, tag="g")
                            nc.tensor.matmul(gpsm[:],
                                             wb_c[:, ft * 128:(ft + 1) * 128],
                                             ghg[:], start=True, stop=True)
                            upsm = ps_up.tile([128, C], f32, tag="up")
                            for k in range(8):
                                nc.tensor.matmul(
                                    upsm[:], wup_c[:, k, ft * 128:(ft + 1) * 128],
                                    x_g[:, k, :], start=(k == 0), stop=(k == 7))
                            sil = sp.tile([128, C], bf16, tag="sil")
                            nc.scalar.activation(sil[:], gpsm[:], AF.Silu)
                            nc.vector.tensor_tensor(hbuf[:, ft, :], sil[:],
                                                    upsm[:], OP.mult)
                        for sc in range(2):
                            for hh in range(2):
                                ya = ps_ya.tile([128, 512], f32, tag="ya")
                                for ft in range(8):
                                    nc.tensor.matmul(
                                        ya[:],
                                        hbuf[:, ft, sc * 128:(sc + 1) * 128],
                                        wdn_c[:, ft, hh * 512:(hh + 1) * 512],
                                        start=(ft == 0), stop=(ft == 7))
                                ysl = y_all[:, e * 2 + sc, hh * 512:(hh + 1) * 512]
                                if fc == 0:
                                    nc.scalar.copy(ysl, ya[:])
                                else:
                                    nc.vector.tensor_tensor(ysl, ysl, ya[:],
                                                            OP.add)

            # ---------------- combine ----------------
            with tc.tile_pool(name="ps_c", bufs=4, space="PSUM") as ps_c:
                for c in range(4):
                    for hh in range(2):
                        cp = ps_c.tile([128, 512], f32, tag="cp")
                        for idx in range(2 * E):
                            nc.tensor.matmul(
                                cp[:], qw[:, idx, c * 128:(c + 1) * 128],
                                y_all[:, idx, hh * 512:(hh + 1) * 512],
                                start=(idx == 0), stop=(idx == 2 * E - 1))
                        nc.scalar.copy(out_sb[:, c, hh * 512:(hh + 1) * 512], cp[:])
                    nc.sync.dma_start(out_d[:, c, :], out_sb[:, c, :])

    nc.compile()
    return nc


def _build_nc():
    import concourse.mybir as mybir
    import concourse.tile as tile
    from concourse import bacc

    f32 = mybir.dt.float32
    bf16 = mybir.dt.bfloat16
    AF = mybir.ActivationFunctionType
    OP = mybir.AluOpType
    AX = mybir.AxisListType

    nc = bacc.Bacc("TRN2", target_bir_lowering=False, debug=False,
                   num_devices=NCORES)

    xT_d = nc.dram_tensor("xT", [128, 8, T], f32, kind="ExternalInput")
    xTbf_d = nc.dram_tensor("xTbf", [128, 8, T], bf16, kind="ExternalInput")
    WA_d = nc.dram_tensor("WA", [128, E, 8, R], f32, kind="ExternalInput")
    WB_d = nc.dram_tensor("WB", [E, 128, F], bf16, kind="ExternalInput")
    WUP_d = nc.dram_tensor("WUP", [E, 128, 8, F], bf16, kind="ExternalInput")
    WDN_d = nc.dram_tensor("WDN", [E, 128, 16, H], bf16, kind="ExternalInput")
    esel_d = nc.dram_tensor("esel", [128, E, E], f32, kind="ExternalInput")
    bsel_d = nc.dram_tensor("bsel", [8, E, 128], f32, kind="ExternalInput")
    ident_d = nc.dram_tensor("ident", [128, 128], f32, kind="ExternalInput")
    out_d = nc.dram_tensor("out", [128, 4, H], f32, kind="ExternalOutput")

    with tile.TileContext(nc) as tc:
        from contextlib import ExitStack
        with ExitStack() as ctx:
            pp = ctx.enter_context(tc.tile_pool(name="persist", bufs=1))

            # persistent SBUF tensors
            xTbf = pp.tile([128, 8, T], bf16, tag="xTbf")
            nc.sync.dma_start(xTbf[:], xTbf_d[:])
            gh_bf = pp.tile([128, E, T], bf16, tag="gh_bf")
            nc.vector.memset(gh_bf[:], 0.0)
            w_bc = pp.tile([128, E, T], bf16, tag="w_bc")
            yT = pp.tile([128, 4, H], f32, tag="yT")
            nc.vector.memset(yT[:], 0.0)

            # ---------------- gate phase ----------------
            with tc.tile_pool(name="gpool", bufs=1) as gp, \
                 tc.tile_pool(name="gpsum", bufs=2, space="PSUM") as gps:
                xTf = gp.tile([128, 8, T], f32, tag="xTf")
                nc.sync.dma_start(xTf[:], xT_d[:])
                wa = gp.tile([128, E, 8, R], f32, tag="wa")
                nc.sync.dma_start(wa[:], WA_d[:])
                esel = gp.tile([128, E, E], f32, tag="esel")
                nc.sync.dma_start(esel[:], esel_d[:])
                bsel = gp.tile([8, E, 128], f32, tag="bsel")
                nc.sync.dma_start(bsel[:], bsel_d[:])
                ident = gp.tile([128, 128], f32, tag="ident")
                nc.sync.dma_start(ident[:], ident_d[:])

                gh2 = gp.tile([128, E, T], f32, tag="gh2")
                nc.vector.memset(gh2[:], 0.0)

                for e in range(E):
                    ghp = gps.tile([64, T], f32, tag="gh")
                    for k in range(8):
                        nc.tensor.matmul(ghp[:], wa[:, e, k, :], xTf[:, k, :],
                                         start=(k == 0), stop=(k == 7))
                    nc.scalar.copy(gh_bf[0:64, e, :], ghp[:])
                    nc.scalar.square(gh2[0:64, e, :], ghp[:])

                # token-major sum of squares: s_tok[t, e] per 128-token chunk
                s_all = gp.tile([128, 4, E], f32, tag="s_all")
                for c in range(4):
                    stp = gps1.tile([128, E], f32, tag="stok")
                    for e in range(E):
                        nc.tensor.matmul(stp[:], gh2[:, e, c * 128:(c + 1) * 128],
                                         esel[:, e, :],
                                         start=(e == 0), stop=(e == E - 1))
                    nc.scalar.sqrt(s_all[:, c, :], stp[:])

                # top-2 + softmax over E=8 per token
                m1 = gp.tile([128, 4], f32, tag="m1")
                nc.vector.reduce_max(m1[:], s_all[:], axis=AX.X)
                m1b = m1[:, :, None].to_broadcast((128, 4, E))
                eqm = gp.tile([128, 4, E], f32, tag="eqm")
                nc.vector.tensor_tensor(eqm[:], s_all[:], m1b, OP.is_ge)
                s2 = gp.tile([128, 4, E], f32, tag="s2")
                nc.vector.scalar_tensor_tensor(s2[:], eqm[:], -1e30, s_all[:],
                                               OP.mult, OP.add)
                m2 = gp.tile([128, 4], f32, tag="m2")
                nc.vector.reduce_max(m2[:], s2[:], axis=AX.X)
                m2b = m2[:, :, None].to_broadcast((128, 4, E))

                d1 = gp.tile([128, 4, E], f32, tag="d1")
                nc.vector.tensor_tensor(d1[:], s_all[:], m1b, OP.subtract)
                e1 = gp.tile([128, 4, E], f32, tag="e1")
                nc.scalar.activation(e1[:], d1[:], AF.Exp)
                dm2 = gp.tile([128, 4], f32, tag="dm2")
                nc.vector.tensor_tensor(dm2[:], m2[:], m1[:], OP.subtract)
                em2 = gp.tile([128, 4], f32, tag="em2")
                nc.scalar.activation(em2[:], dm2[:], AF.Exp)
                den = gp.tile([128, 4], f32, tag="den")
                nc.vector.tensor_scalar_add(den[:], em2[:], 1.0)
                rec = gp.tile([128, 4], f32, tag="rec")
                nc.vector.reciprocal(rec[:], den[:])
                recb = rec[:, :, None].to_broadcast((128, 4, E))
                mask2 = gp.tile([128, 4, E], f32, tag="mask2")
                nc.vector.tensor_tensor(mask2[:], s_all[:], m2b, OP.is_ge)
                wm = gp.tile([128, 4, E], f32, tag="wm")
                nc.vector.tensor_tensor(wm[:], e1[:], mask2[:], OP.mult)
                w_all = gp.tile([128, 4, E], f32, tag="w_all")
                nc.vector.tensor_tensor(w_all[:], wm[:], recb, OP.mult)

                # transpose back: w_all [128t, c, e] -> wT [8e, T]
                wT = gp.tile([8, T], f32, tag="wT")
                for c in range(4):
                    wtp = gps1.tile([8, 128], f32, tag="wtp")
                    nc.tensor.transpose(wtp[:], w_all[:, c, :], ident[:])
                    nc.vector.tensor_copy(wT[:, c * 128:(c + 1) * 128], wtp[:])

                # broadcast across partitions: w_bc[:, e, t] = wT[e, t]
                for e in range(E):
                    wbp = gps.tile([128, T], f32, tag="wbp")
                    nc.tensor.matmul(wbp[:], bsel[:, e, :], wT[:],
                                     start=True, stop=True)
                    nc.scalar.copy(w_bc[:, e, :], wbp[:])

            # ---------------- main expert loop ----------------
            with tc.tile_pool(name="wpool", bufs=2) as wp, \
                 tc.tile_pool(name="xspool", bufs=2) as xsp, \
                 tc.tile_pool(name="hpool", bufs=2) as hp, \
                 tc.tile_pool(name="spool", bufs=3) as sp, \
                 tc.tile_pool(name="psum_mm", bufs=2, space="PSUM") as pmm, \
                 tc.tile_pool(name="psum_y", bufs=4, space="PSUM") as pyy:

                for e in range(E):
                    x_s = xsp.tile([128, 8, T], bf16, tag="x_s")
                    for k in range(8):
                        nc.vector.tensor_tensor(x_s[:, k, :], xTbf[:, k, :],
                                                w_bc[:, e, :], OP.mult)
                    for fc in range(2):
                        wup_c = wp.tile([128, 8, FCH], bf16, tag="wup")
                        nc.sync.dma_start(
                            wup_c[:], WUP_d[e, :, :, fc * FCH:(fc + 1) * FCH])
                        wdn_c = wp.tile([128, 8, H], bf16, tag="wdn")
                        nc.sync.dma_start(
                            wdn_c[:], WDN_d[e, :, fc * 8:(fc + 1) * 8, :])
                        wb_c = wp.tile([128, FCH], bf16, tag="wb")
                        nc.sync.dma_start(
                            wb_c[:], WB_d[e, :, fc * FCH:(fc + 1) * FCH])

                        for tg in range(2):
                            hbuf = hp.tile([128, 8, TG], bf16, tag="h")
                            for ft in range(8):
                                gpsm = pmm.tile([128, TG], f32, tag="g")
                                nc.tensor.matmul(
                                    gpsm[:],
                                    wb_c[:, ft * 128:(ft + 1) * 128],
                                    gh_bf[:, e, tg * TG:(tg + 1) * TG],
                                    start=True, stop=True)
                                upsm = pmm.tile([128, TG], f32, tag="up")
                                for k in range(8):
                                    nc.tensor.matmul(
                                        upsm[:],
                                        wup_c[:, k, ft * 128:(ft + 1) * 128],
                                        x_s[:, k, tg * TG:(tg + 1) * TG],
                                        start=(k == 0), stop=(k == 7))
                                sil = sp.tile([128, TG], bf16, tag="sil")
                                nc.scalar.activation(sil[:], gpsm[:], AF.Silu)
                                nc.vector.tensor_tensor(hbuf[:, ft, :], sil[:],
                                                        upsm[:], OP.mult)
                            # down: token-major output, contract over f
                            for tc2 in range(2):
                                for hh in range(2):
                                    yp = pyy.tile([128, 512], f32, tag="yp")
                                    for ft in range(8):
                                        nc.tensor.matmul(
                                            yp[:],
                                            hbuf[:, ft, tc2 * 128:(tc2 + 1) * 128],
                                            wdn_c[:, ft, hh * 512:(hh + 1) * 512],
                                            start=(ft == 0), stop=(ft == 7))
                                    ysl = yT[:, tg * 2 + tc2,
                                             hh * 512:(hh + 1) * 512]
                                    nc.vector.tensor_tensor(ysl, ysl, yp[:],
                                                            OP.add)

                nc.sync.dma_start(out_d[:], yT[:])

    nc.compile()
    return nc


def _get_nc(impl):
    key = "nc_" + impl
    if key not in _NC_CACHE:
        _NC_CACHE[key] = (_build_nc_sparse() if impl == "sparse" else _build_nc())
    return _NC_CACHE[key]


def _prep_inputs(hidden_states, W_A, W_B, W_up, W_down):
    f32 = np.float32
    x2d = np.ascontiguousarray(np.asarray(hidden_states, dtype=f32).reshape(N, H))
    xT = np.ascontiguousarray(x2d.T)                        # [H, N]
    xT_arr = np.ascontiguousarray(
        xT.reshape(8, 128, N).transpose(1, 0, 2))           # [128, 8, N]
    xTbf_arr = xT_arr.astype(BF16)

    W_A = np.asarray(W_A, dtype=f32)
    W_B = np.asarray(W_B, dtype=f32)
    W_up = np.asarray(W_up, dtype=f32)
    W_down = np.asarray(W_down, dtype=f32)

    # WA: [E,R,H] -> [128, E, 8, R] fp32
    WAh = np.ascontiguousarray(
        W_A.transpose(0, 2, 1).reshape(E, 8, 128, R).transpose(2, 0, 1, 3))
    # WB: [E,F,R] -> [E, 128(R pad), F] bf16
    WBt = W_B.transpose(0, 2, 1)                            # [E, R, F]
    WBh = np.zeros((E, 128, F), dtype=BF16)
    WBh[:, :R, :] = WBt.astype(BF16)
    # WUP: [E,F,H] -> [E, 128, 8, F] bf16   (h = k*128 + p)
    WUPh = np.ascontiguousarray(
        W_up.transpose(0, 2, 1).reshape(E, 8, 128, F).transpose(0, 2, 1, 3)
    ).astype(BF16)
    # WDN: [E,H,F] -> [E, 128, 16, H] bf16  (f = k*128 + p)
    WDNh = np.ascontiguousarray(
        W_down.transpose(0, 2, 1).reshape(E, 16, 128, H).transpose(0, 2, 1, 3)
    ).astype(BF16)

    esel = np.zeros((128, E, E), dtype=f32)
    for e in range(E):
        esel[:R, e, e] = 1.0
    bsel = np.zeros((8, E, 128), dtype=f32)
    for e in range(E):
        bsel[e, e, :] = 1.0
    ident = np.eye(128, dtype=f32)

    shared = dict(WA=WAh, WB=WBh, WUP=WUPh, WDN=WDNh,
                  esel=esel, bsel=bsel, ident=ident)
    in_maps = []
    for c in range(NCORES):
        m = dict(shared)
        m["xT"] = np.ascontiguousarray(xT_arr[:, :, c * T:(c + 1) * T])
        m["xTbf"] = np.ascontiguousarray(xTbf_arr[:, :, c * T:(c + 1) * T])
        in_maps.append(m)
    return in_maps


def _prep_inputs_sparse(hidden_states, W_A, W_B, W_up, W_down):
    f32 = np.float32
    x2d = np.ascontiguousarray(np.asarray(hidden_states, dtype=f32).reshape(N, H))
    xT = np.ascontiguousarray(x2d.T)                        # [H, N]
    xT_arr = np.ascontiguousarray(
        xT.reshape(8, 128, N).transpose(1, 0, 2))           # [128, 8, N]
    # token-major x: [128(t%128... t = c*128+p within core), 4, H]
    xtok_arr = np.ascontiguousarray(
        x2d.reshape(NCORES, 4, 128, H).transpose(0, 2, 1, 3)).astype(BF16)

    W_A = np.asarray(W_A, dtype=f32)
    W_B = np.asarray(W_B, dtype=f32)
    W_up = np.asarray(W_up, dtype=f32)
    W_down = np.asarray(W_down, dtype=f32)

    WA_t = W_A.transpose(0, 2, 1).reshape(E, 8, 128, R)     # [E, k, p, R]
    # pair-packed fp32 gate weights: [128, 4, 8, 128] (cols 0:64 even, 64:128 odd)
    WApk = np.zeros((128, 4, 8, 128), dtype=f32)
    for pr in range(4):
        WApk[:, pr, :, 0:64] = WA_t[2 * pr].transpose(1, 0, 2)
        WApk[:, pr, :, 64:128] = WA_t[2 * pr + 1].transpose(1, 0, 2)
    # bf16 gate weights for gathered recompute: [128, E, 8, R]
    WAbf = np.ascontiguousarray(WA_t.transpose(2, 0, 1, 3)).astype(BF16)

    WBt = W_B.transpose(0, 2, 1)                            # [E, R, F]
    WBh = np.zeros((E, 128, F), dtype=BF16)
    WBh[:, :R, :] = WBt.astype(BF16)
    WUPh = np.ascontiguousarray(
        W_up.transpose(0, 2, 1).reshape(E, 8, 128, F).transpose(0, 2, 1, 3)
    ).astype(BF16)
    WDNh = np.ascontiguousarray(
        W_down.transpose(0, 2, 1).reshape(E, 16, 128, H).transpose(0, 2, 1, 3)
    ).astype(BF16)

    esel = np.zeros((128, 4, E), dtype=f32)
    for pr in range(4):
        esel[0:64, pr, 2 * pr] = 1.0
        esel[64:128, pr, 2 * pr + 1] = 1.0
    bsel = np.zeros((8, E, 128), dtype=f32)
    for e in range(E):
        bsel[e, e, :] = 1.0
    ident = np.eye(128, dtype=f32)
    sbc = np.zeros((128, 2), dtype=f32)
    sbc[:, 0] = np.arange(128)
    sbc[:, 1] = np.arange(128) + 128
    slotbc = np.tile(np.arange(C, dtype=f32)[None, :], (128, 1))

    shared = dict(WApk=WApk, WAbf=WAbf, WB=WBh, WUP=WUPh, WDN=WDNh,
                  esel=esel, bsel=bsel, ident=ident, sbc=sbc, slotbc=slotbc)
    in_maps = []
    for c in range(NCORES):
        m = dict(shared)
        m["xT"] = np.ascontiguousarray(xT_arr[:, :, c * T:(c + 1) * T])
        m["xtok"] = np.ascontiguousarray(xtok_arr[c])
        in_maps.append(m)
    return in_maps


def kernel(hidden_states, W_A, W_B, W_up, W_down):
    global LAST_RESULT
    trace = _maybe_install_trace_hook()
    from concourse import bass_utils

    impl = os.environ.get("MOE_IMPL", "sparse")
    nc = _get_nc(impl)
    if impl == "sparse":
        in_maps = _prep_inputs_sparse(hidden_states, W_A, W_B, W_up, W_down)
    else:
        in_maps = _prep_inputs(hidden_states, W_A, W_B, W_up, W_down)
    res = bass_utils.run_bass_kernel_spmd(
        nc, in_maps, core_ids=list(range(NCORES)), trace=trace)
    LAST_RESULT = res

    out = np.empty((N, H), dtype=np.float32)
    for c in range(NCORES):
        arr = res.results[c]["out"]                        # [128, 4, H]
        out[c * T:(c + 1) * T] = arr.transpose(1, 0, 2).reshape(T, H)
    return out.reshape(B, S, H)
